# revision 89
# baseline (speedup 1.0000x reference)
"""Trainium2 Bass kernel for nn_Block (attention + soft top-2 MoE), 8-core SPMD.

Sharding:
  - Attention: stride-4 interleaved causal balance.  Core c = 4b+l owns
    batch-b tokens t === l (mod 4), sorted DESCENDING into its 512 qT
    columns, so key slot s (tokens [128s,128s+128)) is needed by exactly
    the first max(256, 512-32s) columns on EVERY core (uniform SPMD
    schedule, ~50% less score/exp work than the rectangular form).  K/V
    are AllGathered in two d-halves (merged k+v buffers; the second AG is
    triggered after the first pass's gathers so it hides behind wave
    compute); causal staircase masks come from the host.  The per-query
    gate bias is constant along the softmax axis, so it cancels.
  - Router logits run in TRUE f32 (fp22-truncated f32r flips near-tied
    top-2 tokens; min prob margin here is 4.5e-6).
  - MoE: expert-parallel, DENSE — every core runs its expert's fp8
    DoubleRow FFN over all 4096 tokens and scales each token's output by
    its routing weight (0 if not routed here); a bf16 ReduceScatter sums
    the <=2 live contributions per token.  No compaction/indirect DMA.
  - The identity "dummy" expert is applied locally by each token's owner.
"""

import sys

if "/opt/trn_rl_repo" not in sys.path:
    sys.path.insert(0, "/opt/trn_rl_repo")

import numpy as np

import concourse.bass as bass
import concourse.mybir as mybir
import concourse.tile as tile
from concourse import bacc
from concourse.masks import make_identity

F32 = mybir.dt.float32
F32R = mybir.dt.float32r
BF16 = mybir.dt.bfloat16
FP8 = mybir.dt.float8e4
I32 = mybir.dt.int32
AF = mybir.ActivationFunctionType
ALU = mybir.AluOpType
DR = mybir.MatmulPerfMode.DoubleRow
W8SCALE = 256.0

B, T, D = 2, 2048, 1024
H, HD = 16, 64
E = 8
NC = 8
N = B * T                  # 4096 tokens
OWN = N // NC              # 512 tokens per core
NCH = 16                   # key slots (128 keys each, token-sorted)
DFF = 4 * D
CAP = 1024                 # expert token capacity per core
VA = 65                    # v columns per head (64 + ones)
EPS = 1e-5
RG = [list(range(NC))]
HG = 4                     # heads per attention head-group

# Balanced-causal slot schedule.  Queries are owned stride-4 interleaved
# (core l of a batch owns tokens t≡l mod 4) and sorted DESCENDING by
# position in qT columns, so slot s (keys [128s,128s+128)) is needed
# exactly by the first 512-32s columns on EVERY core.  Width is padded
# to >=256 to keep f32r matmuls at 1 cycle/row; the mask region covers
# the causal staircase plus the padding.
SLOT_W = []                # (width, mask_start, mask_width, mask_off)
_moff = 0
for _s in range(NCH):
    _wt = 512 - 32 * _s
    _ws = max(256, _wt)
    _st = max(0, _wt - 32)
    SLOT_W.append((_ws, _st, _ws - _st, _moff))
    _moff += _ws - _st
MTOT = _moff               # 1408 mask columns


def r32(x):
    return x.bitcast(F32R)


def build_nc():
    nc = bacc.Bacc("TRN2", target_bir_lowering=False, debug=False,
                   num_devices=NC)

    # ---- I/O ----
    x_own = nc.dram_tensor("x_own", [OWN, D], F32, kind="ExternalInput")
    wqkv = nc.dram_tensor("wqkv", [D, 3 * D], F32R, kind="ExternalInput")
    wproj = nc.dram_tensor("wproj", [D, D], F32R, kind="ExternalInput")
    wrouter = nc.dram_tensor("wrouter", [D, E + 1], F32, kind="ExternalInput")
    wfc8 = nc.dram_tensor("wfc8", [128, 32 * 8 * 128], FP8,
                          kind="ExternalInput")
    wpj8 = nc.dram_tensor("wpj8", [128, 16 * 2 * 1024], FP8,
                          kind="ExternalInput")
    ln2bc = nc.dram_tensor("ln2bc", [128, D], F32, kind="ExternalInput")
    dmask = nc.dram_tensor("dmask", [128, MTOT], F32R, kind="ExternalInput")
    ksel = nc.dram_tensor("ksel", [128, NCH], I32, kind="ExternalInput")
    emask = nc.dram_tensor("emask", [128, 16], F32, kind="ExternalInput")
    out = nc.dram_tensor("out", [OWN, D], F32, kind="ExternalOutput")

    # ---- internal DRAM ----
    agin_kv = [nc.dram_tensor(f"agin_kv{i}", [OWN, 512 + 8 * VA], F32)
               for i in range(2)]
    agout_kv = [nc.dram_tensor(f"agout_kv{i}", [N, 512 + 8 * VA], F32,
                               addr_space="Shared") for i in range(2)]
    agin_h2 = nc.dram_tensor("agin_h2", [OWN, D + 16], BF16)
    agout_h2 = nc.dram_tensor("agout_h2", [N, D + 16], BF16,
                              addr_space="Shared")
    xmid_d = nc.dram_tensor("xmid_d", [OWN, D], F32)
    yt2_d = nc.dram_tensor("yt2_d", [128, 8 * OWN], F32R)
    rsin = nc.dram_tensor("rsin", [N, D], BF16)
    rsout = nc.dram_tensor("rsout", [OWN, D], BF16)

    with tile.TileContext(nc) as tc:
        build_body(nc, tc, locals())
    nc.compile()
    return nc


def build_body(nc, tc, t):
    x_own, wqkv, wproj, wrouter, wfc8, wpj8 = (
        t["x_own"], t["wqkv"], t["wproj"], t["wrouter"], t["wfc8"], t["wpj8"])
    ln2bc, dmask, ksel, emask = (
        t["ln2bc"], t["dmask"], t["ksel"], t["emask"])
    agin_kv, agout_kv = t["agin_kv"], t["agout_kv"]
    agin_h2, agout_h2 = t["agin_h2"], t["agout_h2"]
    xmid_d, yt2_d, rsin, rsout, out = (
        t["xmid_d"], t["yt2_d"], t["rsin"], t["rsout"], t["out"])

    ident_p = tc.alloc_tile_pool(name="ident", bufs=1)
    ident = ident_p.tile([128, 128], F32)
    make_identity(nc, ident[:])

    cst_p = tc.alloc_tile_pool(name="cst", bufs=1)
    ksel_sb = cst_p.tile([128, NCH], I32)
    nc.sync.dma_start(ksel_sb[:], ksel[:, :])
    emask_sb = cst_p.tile([128, 16], F32)
    nc.sync.dma_start(emask_sb[:], emask[:, :])
    ones_cf = cst_p.tile([128, 1], F32)
    nc.vector.memset(ones_cf[:], 1.0)
    ones_c = cst_p.tile([128, 1], F32R)
    nc.scalar.activation(ones_c[:], ones_cf[:], AF.Copy)
    ones_rf = cst_p.tile([1, 128], F32)
    nc.vector.memset(ones_rf[:], 1.0)
    ones_r = cst_p.tile([1, 128], F32R)
    nc.scalar.activation(ones_r[:], ones_rf[:], AF.Copy)
    ones33f = cst_p.tile([33, 64], F32)
    nc.vector.memset(ones33f[:], 1.0)
    ones33 = cst_p.tile([33, 64], F32R)
    nc.scalar.activation(ones33[:], ones33f[:], AF.Copy)
    eps_sb = cst_p.tile([128, 1], F32)
    nc.vector.memset(eps_sb[:], EPS)
    nc.eps_sb = eps_sb
    w8 = cst_p.tile([128, 4], F32)              # identity-expert weight
    wff_p = tc.alloc_tile_pool(name="wff", bufs=1)
    wfcs = wff_p.tile([128, 32, 8, 128], FP8)

    # ------- Phase 1: LN1 + QKV; K/V in d-halves with early AllGathers ----
    ab_p = tc.alloc_tile_pool(name="pAB", bufs=1)
    qT = ab_p.tile([128, 8, OWN], F32R)          # [qdim, mc, tok] (x 1/8)
    with tc.tile_pool(name="p1", bufs=2) as p1, \
         tc.tile_pool(name="p1w", bufs=2) as p1w:
        p1ps_cm = tc.tile_pool(name="p1psA", bufs=2, space="PSUM")
        p1ps = p1ps_cm.__enter__()
        p1kv_cm = tc.tile_pool(name="p1psKV", bufs=1, space="PSUM")
        p1kv = p1kv_cm.__enter__()
        xlnT = p1.tile([128, 8, OWN], F32R, tag="xlnT", bufs=1)
        for tt in range(4):
            xs = p1.tile([128, D], F32, tag="xs")
            nc.sync.dma_start(xs[:], x_own[tt * 128:(tt + 1) * 128, :])
            xln = _layernorm(nc, p1, xs, D)
            for dt in range(8):
                pst = p1ps.tile([128, 128], F32, tag="tp")
                nc.tensor.transpose(pst[:], xln[:, dt * 128:(dt + 1) * 128],
                                    ident[:])
                nc.scalar.activation(xlnT[:, dt, tt * 128:(tt + 1) * 128],
                                     pst[:], AF.Copy)
        # token-major k and v(+ones), one d-half at a time; the half-0 AG
        # fires here, the half-1 AG is triggered inside phase 2 (after the
        # pass-0 gathers) so it hides behind wave compute.
        pks = [p1kv.tile([128, 512], F32, tag=f"pkv{i}", name=f"pkv{i}")
               for i in range(4)]
        for half in range(2):
            for sec in range(2):  # 0 = k, 1 = v
                for dt in range(8):
                    wp = p1w.tile([128, 512], F32R, tag="wkv")
                    base = (1 + sec) * D + half * 512
                    nc.sync.dma_start(
                        wp[:], wqkv[dt * 128:(dt + 1) * 128,
                                    base:base + 512])
                    for tt in range(4):
                        nc.tensor.matmul(
                            pks[tt][:],
                            r32(xlnT[:, dt, tt * 128:(tt + 1) * 128]),
                            r32(wp[:]), start=(dt == 0), stop=(dt == 7))
                for tt in range(4):
                    if sec == 0:
                        ks = p1.tile([128, 512], F32, tag="ko")
                        nc.vector.tensor_copy(ks[:], pks[tt][:])
                        nc.sync.dma_start(
                            agin_kv[half][tt * 128:(tt + 1) * 128, 0:512],
                            ks[:])
                    else:
                        vs = p1.tile([128, 8 * VA], F32, tag="vo")
                        vv = vs[:].rearrange("p (h c) -> p h c", c=VA)
                        nc.vector.memset(vv[:, :, 64:65], 1.0)
                        nc.scalar.activation(
                            vv[:, :, 0:64],
                            pks[tt][:].rearrange("p (h c) -> p h c", c=64),
                            AF.Copy)
                        nc.sync.dma_start(
                            agin_kv[half][tt * 128:(tt + 1) * 128,
                                          512:512 + 8 * VA], vs[:])
            if half == 0:
                nc.gpsimd.collective_compute(
                    "AllGather", ALU.bypass, replica_groups=RG,
                    ins=[agin_kv[0][:, :].opt()],
                    outs=[agout_kv[0][:, :].opt()])
        p1kv_cm.__exit__(None, None, None)
        # qT: lhsT = Wq block -> direct [qdim, tok] layout; scaled 1/sqrt(HD)
        for mc in range(8):
            wp = p1w.tile([128, 8, 128], F32R, tag="wq")
            nc.sync.dma_start(
                wp[:], wqkv[:, mc * 128:(mc + 1) * 128].rearrange(
                    "(dt p) m -> p dt m", p=128))
            psq = p1ps.tile([128, OWN], F32, tag="psq")
            for dt in range(8):
                nc.tensor.matmul(
                    psq[:], r32(wp[:, dt, :]), r32(xlnT[:, dt, :]),
                    start=(dt == 0), stop=(dt == 7))
            nc.scalar.activation(qT[:, mc, :], psq[:], AF.Copy, scale=0.125)
        p1ps_cm.__exit__(None, None, None)

    # FFN weights prefetched on the sync queue; they stream in under the
    # K/V AllGathers and attention waves.
    nc.sync.dma_start(
        wfcs[:], wfc8[:, :].rearrange("p (g dt m) -> p g dt m",
                                      dt=8, m=128))

    # -------- Phase 2: attention (balanced causal slots, two passes) ------
    with tc.tile_pool(name="p2", bufs=1) as p2, \
         tc.tile_pool(name="p2s", bufs=3) as p2s:
        dm_sb = p2.tile([128, MTOT], F32R, tag="dm")
        nc.sync.dma_start(dm_sb[:], dmask[:, :])
        for pz in range(2):     # pass 0: heads 0-7, pass 1: heads 8-15
            kTh = p2.tile([128, NCH, 4, 128], F32R, tag="kTh", bufs=1,
                          name="kTh")
            kvs = []
            tp_cm = tc.tile_pool(name=f"p2psT{pz}", bufs=2, space="PSUM")
            tpp = tp_cm.__enter__()
            for s in range(NCH):
                kvch = p2.tile([128, 512 + 8 * VA], F32R, tag=f"kvch{s}",
                               bufs=1, name=f"kvch{s}")
                kvs.append(kvch)
                nc.gpsimd.indirect_dma_start(
                    out=kvch[:], out_offset=None, in_=agout_kv[pz][:, :],
                    in_offset=bass.IndirectOffsetOnAxis(
                        ap=ksel_sb[:, s:s + 1], axis=0))
                for dtl in range(4):
                    pst = tpp.tile([128, 128], F32, tag="tp2")
                    nc.tensor.transpose(
                        pst[:], kvch[:, dtl * 128:(dtl + 1) * 128].bitcast(
                            F32), ident[:])
                    nc.vector.tensor_copy(kTh[:, s, dtl, :], pst[:])
            if pz == 0:
                # half-1 AllGather rides behind the pass-0 waves
                nc.gpsimd.collective_compute(
                    "AllGather", ALU.bypass, replica_groups=RG,
                    ins=[agin_kv[1][:, :].opt()],
                    outs=[agout_kv[1][:, :].opt()])
            tp_cm.__exit__(None, None, None)
            sc_cm = tc.tile_pool(name=f"p2psS{pz}", bufs=2, space="PSUM")
            scp = sc_cm.__enter__()
            yp_cm = tc.tile_pool(name=f"p2psY{pz}", bufs=1, space="PSUM")
            ypp = yp_cm.__enter__()
            for wave in ([0, 1, 2, 3, 4], [5, 6, 7]):
                ypss = {}
                for wi, hh in enumerate(wave):
                    ypss[hh] = ypp.tile([VA, OWN], F32, tag=f"yps{wi}",
                                        name=f"yps{wi}")
                def do_av(s2, hh2, ex2):
                    ws2 = SLOT_W[s2][0]
                    nc.tensor.matmul(
                        ypss[hh2][:, 0:ws2],
                        kvs[s2][:, 512 + hh2 * VA:512 + (hh2 + 1) * VA],
                        r32(ex2[:, 0:ws2]),
                        start=(s2 == 0), stop=(s2 == NCH - 1),
                        skip_group_check=True)

                # lag-2 software pipeline keeps the PE queue free of
                # exp-dependency stalls (f32r pays 2x when the PE cools)
                pend = []
                for s in range(NCH):
                    ws, mst, mw, moff = SLOT_W[s]
                    for hh in wave:
                        dtl, ph = hh // 2, (hh % 2) * 64
                        ssc = scp.tile([128, OWN], F32, tag="ssc")
                        nc.tensor.matmul(
                            ssc[:, 0:ws],
                            r32(kTh[ph:ph + 64, s, dtl, :]),
                            r32(qT[ph:ph + 64, 4 * pz + dtl, 0:ws]),
                            start=True, stop=True)
                        ex = p2s.tile([128, OWN], F32R, tag="ex")
                        # exp only the true causal width; the padded tail
                        # [wt:ws] keeps stale (finite) values that the mask
                        # multiply zeroes exactly
                        wt = max(32, 512 - 32 * s)
                        nc.scalar.activation(ex[:, 0:wt], ssc[:, 0:wt],
                                             AF.Exp)
                        nc.vector.tensor_mul(ex[:, mst:mst + mw],
                                             ex[:, mst:mst + mw],
                                             dm_sb[:, moff:moff + mw])
                        pend.append((s, hh, ex))
                        if len(pend) > 2:
                            do_av(*pend.pop(0))
                for item in pend:
                    do_av(*item)
                # softmax normalize + write yT2 (pair-batched reciprocals)
                prs = [wave[i:i + 2] for i in range(0, len(wave), 2)]
                rins = {}
                for pi, pr in enumerate(prs):
                    dsb = p2s.tile([33, OWN], F32, tag=f"dsb{pi}", bufs=1,
                                   name=f"dsb{pi}")
                    nc.vector.memset(dsb[:], 1.0)
                    nc.vector.tensor_copy(dsb[0:1, :], ypss[pr[0]][64:65, :])
                    if len(pr) > 1:
                        nc.vector.tensor_copy(dsb[32:33, :],
                                              ypss[pr[1]][64:65, :])
                    rin2 = p2s.tile([33, OWN], F32R, tag=f"rin2{pi}", bufs=1,
                                    name=f"rin2{pi}")
                    with nc.allow_low_precision(reason="f32r bcast rhs"):
                        nc.vector.reciprocal(rin2[:], dsb[:])
                    rins[pr[0]] = (rin2, 0)
                    if len(pr) > 1:
                        rins[pr[1]] = (rin2, 32)
                for hh in wave:
                    h = pz * 8 + hh
                    mc, ph = h // 2, (h % 2) * 64
                    rin2, rb = rins[hh]
                    pbc = scp.tile([128, OWN], F32, tag="ssc")
                    nc.tensor.matmul(pbc[0:64, :],
                                     r32(ones33[rb:rb + 1, :]),
                                     r32(rin2[rb:rb + 1, :]),
                                     start=True, stop=True)
                    pbs = p2s.tile([64, OWN], F32, tag="pbs")
                    nc.scalar.activation(pbs[:], pbc[0:64, :], AF.Copy)
                    yo = p2s.tile([64, OWN], F32R, tag="yo")
                    nc.vector.tensor_mul(yo[:], ypss[hh][0:64, :], pbs[:])
                    nc.sync.dma_start(
                        yt2_d[ph:ph + 64, mc * OWN:(mc + 1) * OWN], yo[:])
            yp_cm.__exit__(None, None, None)
            sc_cm.__exit__(None, None, None)
    ab_p.release()

    # ------------- Phase 3: proj + residual + LN2 + router -------------
    with tc.tile_pool(name="p3", bufs=2) as p3, \
         tc.tile_pool(name="p3w", bufs=2) as p3w:
        p3ps_cm = tc.tile_pool(name="p3psA", bufs=2, space="PSUM")
        p3ps = p3ps_cm.__enter__()
        yT2 = p3.tile([128, 8 * OWN], F32R, tag="yT2")
        nc.sync.dma_start(yT2[:], yt2_d[:, :])
        h2nT = p3.tile([128, 8 * OWN], F32, tag="h2nT")
        for tp in range(2):        # token-chunk pairs: one wproj pass each
            pps = [p3ps.tile([128, D], F32, tag=f"pp{i}", bufs=1,
                             name=f"pp{i}") for i in range(2)]
            for dt in range(8):
                wp = p3w.tile([128, D], F32R, tag="wpj3")
                nc.sync.dma_start(wp[:], wproj[dt * 128:(dt + 1) * 128, :])
                for i in range(2):
                    tt = 2 * tp + i
                    for half in range(2):
                        nc.tensor.matmul(
                            pps[i][:, half * 512:(half + 1) * 512],
                            r32(yT2[:, dt * OWN + tt * 128:
                                    dt * OWN + (tt + 1) * 128]),
                            r32(wp[:, half * 512:(half + 1) * 512]),
                            start=(dt == 0), stop=(dt == 7))
            for i in range(2):
                tt = 2 * tp + i
                xot = p3.tile([128, D], F32, tag="xot")
                nc.sync.dma_start(xot[:], x_own[tt * 128:(tt + 1) * 128, :])
                xmt = p3.tile([128, D], F32, tag="xmt")
                nc.vector.tensor_add(xmt[:], xot[:], pps[i][:])
                nc.sync.dma_start(xmid_d[tt * 128:(tt + 1) * 128, :], xmt[:])
                h2t = _layernorm(nc, p3, xmt, D)
                h2b = p3.tile([128, D], BF16, tag="h2b")
                nc.scalar.activation(h2b[:], h2t[:], AF.Copy)
                nc.sync.dma_start(agin_h2[tt * 128:(tt + 1) * 128, 0:D],
                                  h2b[:])
                for dt in range(8):
                    pst = p3ps.tile([128, 128], F32, tag="tp3")
                    nc.tensor.transpose(
                        pst[:], h2t[:, dt * 128:(dt + 1) * 128], ident[:])
                    nc.scalar.activation(
                        h2nT[:, dt * OWN + tt * 128:
                             dt * OWN + (tt + 1) * 128],
                        pst[:], AF.Copy)
        p3ps_cm.__exit__(None, None, None)
        p3ps_cm2 = tc.tile_pool(name="p3psB", bufs=2, space="PSUM")
        p3ps = p3ps_cm2.__enter__()
        # router logitsT [9, 512]
        wr = p3.tile([128, 8 * (E + 1)], F32, tag="wr")
        nc.sync.dma_start(
            wr[:].rearrange("p (dt m) -> p dt m", m=E + 1),
            wrouter[:, :].rearrange("(dt p) m -> p dt m", p=128))
        plg = p3ps.tile([E + 1, OWN], F32, tag="plg")
        for dt in range(8):
            nc.tensor.matmul(
                plg[:], wr[:, dt * (E + 1):(dt + 1) * (E + 1)],
                h2nT[:, dt * OWN:(dt + 1) * OWN],
                start=(dt == 0), stop=(dt == 7))
        lgT = p3.tile([E + 1, OWN], F32, tag="lgT")
        nc.scalar.activation(lgT[:], plg[:], AF.Copy)
        for tt in range(4):
            plt = p3ps.tile([128, E + 1], F32, tag="plt")
            nc.tensor.transpose(plt[:], lgT[:, tt * 128:(tt + 1) * 128],
                                ident[0:E + 1, 0:E + 1])
            # softmax + top-2 weights on [128, 9]
            lg = p3.tile([128, E + 1], F32, tag="lg")
            nc.vector.tensor_copy(lg[:], plt[:])
            rmax = p3.tile([128, 1], F32, tag="rmax")
            nc.vector.reduce_max(rmax[:], lg[:], axis=mybir.AxisListType.X)
            nrm = p3.tile([128, 1], F32, tag="nrm")
            nc.vector.tensor_scalar_mul(nrm[:], rmax[:], -1.0)
            prob = p3.tile([128, E + 1], F32, tag="prob")
            sume = p3.tile([128, 1], F32, tag="sume")
            nc.scalar.activation(prob[:], lg[:], AF.Exp, bias=nrm[:],
                                 accum_out=sume[:])
            rinv = p3.tile([128, 1], F32, tag="rinv")
            nc.vector.reciprocal(rinv[:], sume[:])
            nc.scalar.activation(prob[:], prob[:], AF.Copy, scale=rinv[:])
            m1 = p3.tile([128, 1], F32, tag="m1")
            nc.vector.reduce_max(m1[:], prob[:], axis=mybir.AxisListType.X)
            eq = p3.tile([128, E + 1], F32, tag="eq")
            nc.vector.tensor_tensor(
                out=eq[:], in0=prob[:], in1=m1[:].to_broadcast([128, E + 1]),
                op=ALU.is_equal)
            pm = p3.tile([128, E + 1], F32, tag="pm")
            nc.vector.tensor_scalar_mul(pm[:], eq[:], -2.0)
            nc.vector.tensor_add(pm[:], pm[:], prob[:])
            m2 = p3.tile([128, 1], F32, tag="m2")
            nc.vector.reduce_max(m2[:], pm[:], axis=mybir.AxisListType.X)
            ge = p3.tile([128, E + 1], F32, tag="ge")
            nc.vector.tensor_tensor(
                out=ge[:], in0=prob[:], in1=m2[:].to_broadcast([128, E + 1]),
                op=ALU.is_ge)
            w16 = p3.tile([128, 16], F32, tag="w16")
            nc.vector.memset(w16[:], 0.0)
            nc.vector.tensor_mul(w16[:, 0:E + 1], prob[:], ge[:])
            nc.vector.tensor_copy(w8[:, tt:tt + 1], w16[:, E:E + 1])
            w16b = p3.tile([128, 16], BF16, tag="w16b")
            nc.vector.tensor_copy(w16b[:], w16[:])
            nc.sync.dma_start(agin_h2[tt * 128:(tt + 1) * 128, D:D + 16],
                              w16b[:])
        p3ps_cm2.__exit__(None, None, None)
    nc.gpsimd.collective_compute(
        "AllGather", ALU.bypass, replica_groups=RG,
        ins=[agin_h2[:, :].opt()], outs=[agout_h2[:, :].opt()])

    # ------- Phase 4/5: dense masked expert FFN (fp8 DoubleRow) ----------
    # Every core runs ITS expert over all N tokens and scales each token's
    # output by that expert's routing weight (0 for tokens not routed here);
    # the ReduceScatter then sums the <=2 live expert contributions.  No
    # compaction, no indirect DMA.
    with tc.tile_pool(name="p5g", bufs=3) as p5g, \
         tc.tile_pool(name="p5", bufs=1) as p5, \
         tc.tile_pool(name="p5at", bufs=1) as p5at:
        identb = p5.tile([128, 128], BF16, tag="identb")
        nc.scalar.activation(identb[:], ident[:], AF.Copy)
        # routing weights for this expert: wcol[p, f] (token p+128f), and
        # fold in the 1/W8SCALE fp8 descale
        wfull = p5.tile([128, 32, 16], BF16, tag="wfull")
        nc.sync.dma_start(
            wfull[:],
            agout_h2[:, D:D + 16].rearrange("(f p) c -> p f c", p=128))
        wsel = p5.tile([128, 32, 16], F32, tag="wsel")
        nc.vector.tensor_tensor(
            out=wsel[:], in0=wfull[:],
            in1=emask_sb[:].rearrange("p (o c) -> p o c", o=1).to_broadcast(
                [128, 32, 16]),
            op=ALU.mult)
        wcol = p5.tile([128, 32], F32, tag="wcol")
        nc.vector.reduce_sum(wcol[:], wsel[:], axis=mybir.AxisListType.X)
        nc.vector.tensor_scalar_mul(wcol[:], wcol[:], 1.0 / W8SCALE)
        wpp = []
        for fp in range(16):
            w2 = p5at.tile([128, 2, D], FP8, tag=f"wpp{fp}",
                           name=f"wpp{fp}")
            nc.sync.dma_start(
                w2[:], wpj8[:, fp * 2048:(fp + 1) * 2048].rearrange(
                    "p (s m) -> p s m", m=1024))
            wpp.append(w2)
        at2 = [p5at.tile([128, 2, CAP], FP8, tag=f"at2_{i}",
                         name=f"at2_{i}") for i in range(16)]
        for q4 in range(4):            # token quarters of 1024
            p5ps_cm = tc.tile_pool(name=f"p5psT{q4}", bufs=2, space="PSUM")
            p5ps = p5ps_cm.__enter__()
            h2cT = p5.tile([128, 8, CAP], FP8, tag="h2cT", bufs=2)
            for j in range(8):
                hc = p5g.tile([128, D], BF16, tag="hc")
                nc.sync.dma_start(
                    hc[:],
                    agout_h2[(q4 * 8 + j) * 128:(q4 * 8 + j + 1) * 128, 0:D])
                for dt in range(8):
                    pst = p5ps.tile([128, 128], BF16, tag="tp5")
                    nc.tensor.transpose(
                        pst[:], hc[:, dt * 128:(dt + 1) * 128], identb[:])
                    nc.scalar.activation(h2cT[:, dt, j * 128:(j + 1) * 128],
                                         pst[:], AF.Copy)
            p5ps_cm.__exit__(None, None, None)
            p5ps_cm2 = tc.tile_pool(name=f"p5psB{q4}", bufs=2, space="PSUM")
            p5ps = p5ps_cm2.__enter__()
            for gfc in range(32):
                ps1 = p5ps.tile([128, CAP], F32, tag="ps1")
                for t2 in range(4):
                    for hf in range(2):
                        nc.tensor.matmul(
                            ps1[:, hf * 512:(hf + 1) * 512],
                            wfcs[:, gfc, 2 * t2:2 * t2 + 2, :],
                            h2cT[:, 2 * t2:2 * t2 + 2,
                                 hf * 512:(hf + 1) * 512],
                            start=(t2 == 0), stop=(t2 == 3), perf_mode=DR)
                nc.scalar.activation(at2[gfc // 2][:, gfc % 2, :], ps1[:],
                                     AF.Gelu, scale=1.0 / W8SCALE)
            p5ps_cm2.__exit__(None, None, None)
            p5ps_cm3 = tc.tile_pool(name=f"p5psC{q4}", bufs=2, space="PSUM")
            p5ps3 = p5ps_cm3.__enter__()
            for tt in range(8):
                ps2 = p5ps3.tile([128, D], F32, tag="ps2")
                for fp in range(16):
                    for hf in range(2):
                        nc.tensor.matmul(
                            ps2[:, hf * 512:(hf + 1) * 512],
                            at2[fp][:, :, tt * 128:(tt + 1) * 128],
                            wpp[fp][:, :, hf * 512:(hf + 1) * 512],
                            start=(fp == 0), stop=(fp == 15), perf_mode=DR)
                sc = p5g.tile([128, D], BF16, tag="sc")
                nc.scalar.activation(sc[:], ps2[:], AF.Copy,
                                     scale=wcol[:, q4 * 8 + tt:
                                                q4 * 8 + tt + 1])
                nc.sync.dma_start(
                    rsin[(q4 * 8 + tt) * 128:(q4 * 8 + tt + 1) * 128, :],
                    sc[:])
            p5ps_cm3.__exit__(None, None, None)

    # ---------------- Phase 6: combine via ReduceScatter ----------------
    nc.gpsimd.collective_compute(
        "ReduceScatter", ALU.add, replica_groups=RG,
        ins=[rsin[:, :].opt()], outs=[rsout[:, :].opt()])

    # ---------------- Phase 7: final assembly ----------------
    with tc.tile_pool(name="p7", bufs=2) as p7:
        lnb = p7.tile([128, D], F32, tag="lnb")
        nc.sync.dma_start(lnb[:], ln2bc[:, :])
        for tt in range(4):
            rs = p7.tile([128, D], BF16, tag="rs")
            nc.sync.dma_start(rs[:], rsout[tt * 128:(tt + 1) * 128, :])
            h2t7 = p7.tile([128, D], BF16, tag="h2t7")
            nc.sync.dma_start(h2t7[:],
                              agin_h2[tt * 128:(tt + 1) * 128, 0:D])
            xm7 = p7.tile([128, D], F32, tag="xm7")
            nc.sync.dma_start(xm7[:], xmid_d[tt * 128:(tt + 1) * 128, :])
            idt = p7.tile([128, D], F32, tag="idt")
            nc.vector.tensor_mul(idt[:], h2t7[:], lnb[:])
            nc.scalar.activation(idt[:], idt[:], AF.Copy,
                                 scale=w8[:, tt:tt + 1])
            nc.vector.tensor_add(idt[:], idt[:], rs[:])
            nc.vector.tensor_add(idt[:], idt[:], xm7[:])
            nc.sync.dma_start(out[tt * 128:(tt + 1) * 128, :], idt[:])
    for pl in (wff_p, cst_p, ident_p):
        pl.release()


def _layernorm(nc, pool, xs, d):
    """LN (no weight) on a [128, d] token-major tile; returns a new tile."""
    rsum = pool.tile([128, 1], F32, tag="ln_rsum")
    nc.vector.reduce_sum(rsum[:], xs[:], axis=mybir.AxisListType.X)
    nmean = pool.tile([128, 1], F32, tag="ln_nmean")
    nc.vector.tensor_scalar_mul(nmean[:], rsum[:], -1.0 / d)
    xc = pool.tile([128, d], F32, tag="ln_xc")
    nc.vector.tensor_scalar_add(xc[:], xs[:], nmean[:])
    ssum = pool.tile([128, 1], F32, tag="ln_ssum")
    nc.scalar.activation(xs[:], xc[:], AF.Square, accum_out=ssum[:])
    std = pool.tile([128, 1], F32, tag="ln_std")
    nc.scalar.activation(std[:], ssum[:], AF.Sqrt, bias=nc.eps_sb[:],
                         scale=1.0 / d)
    rstd = pool.tile([128, 1], F32, tag="ln_rstd")
    nc.vector.reciprocal(rstd[:], std[:])
    xo = pool.tile([128, d], F32, tag="ln_xo")
    nc.scalar.activation(xo[:], xc[:], AF.Copy, scale=rstd[:])
    return xo


# ---------------------------------------------------------------------------
# host side
# ---------------------------------------------------------------------------

def _host_prep(inputs):
    """Build per-core in_maps (all numpy, fp32/int32/fp8)."""
    import ml_dtypes
    f8 = ml_dtypes.float8_e4m3

    x = np.asarray(inputs["x"], np.float32).reshape(N, D)
    ln1 = np.asarray(inputs["ln1_w"], np.float32)
    ln2 = np.asarray(inputs["ln2_w"], np.float32)
    wqkv = (np.asarray(inputs["Wqkv"], np.float32) * ln1[:, None]).copy()
    wproj = np.ascontiguousarray(np.asarray(inputs["Wproj"], np.float32))
    wrouter = (np.asarray(inputs["router_W"], np.float32)
               * ln2[:, None]).copy()
    wfc = np.asarray(inputs["W_fc"], np.float32) * ln2[None, :, None]
    wpj = np.asarray(inputs["W_pj"], np.float32)

    def q8(w):
        return np.clip(w * W8SCALE, -240.0, 240.0).astype(f8)

    # wfc8[e][p, gfc*1024 + dt*128 + m] = wfc[e][dt*128+p, gfc*128+m] * 256
    wfc8 = q8(wfc).reshape(E, 8, 128, 32, 128).transpose(0, 2, 3, 1, 4) \
        .reshape(E, 128, 32 * 8 * 128)
    # wpj8[e][p, fp*2048 + s*1024 + m] = wpj[e][fp*256+s*128+p, m] * 256
    wpj8 = q8(wpj).reshape(E, 16, 2, 128, D).transpose(0, 3, 1, 2, 4) \
        .reshape(E, 128, 16 * 2 * 1024)
    ln2bc = np.broadcast_to(ln2, (128, D)).copy()

    # stride-4 interleaved ownership, queries sorted descending:
    # core c = 4b+l owns batch-b tokens t = 2044 + l - 4j  (j = 0..511)
    jj = np.arange(OWN)
    kk = np.arange(128)[:, None]
    # ksel[k, s] = agout row of batch token 128s+k (same for both batches)
    ktok = 128 * np.arange(NCH)[None, :] + kk          # [128, NCH]
    kown = ktok % 4
    krow = (2044 + kown - ktok) // 4
    ksel_all = (kown * OWN + krow).astype(np.int32)

    in_maps = []
    for c in range(NC):
        b, l = c // 4, c % 4
        tj = 2044 + l - 4 * jj                          # [OWN]
        x_own = x[b * T + tj]
        # causal staircase masks per slot, concatenated mask regions
        dmask = np.zeros((128, MTOT), np.float32)
        for s in range(NCH):
            ws, mst, mw, moff = SLOT_W[s]
            cols = tj[mst:mst + mw][None, :]            # query token ids
            keys = 128 * s + kk                         # [128, 1]
            dmask[:, moff:moff + mw] = (cols >= keys).astype(np.float32)
        em = np.zeros((128, 16), np.float32)
        em[:, c] = 1.0
        in_maps.append({
            "x_own": np.ascontiguousarray(x_own),
            "wqkv": wqkv, "wproj": wproj, "wrouter": wrouter,
            "wfc8": np.ascontiguousarray(wfc8[c]),
            "wpj8": np.ascontiguousarray(wpj8[c]),
            "ln2bc": ln2bc, "dmask": dmask,
            "ksel": ksel_all + np.int32(b * 4 * OWN), "emask": em,
        })
    return in_maps


def _host_assemble(results):
    full = np.empty((N, D), np.float32)
    jj = np.arange(OWN)
    for c in range(NC):
        b, l = c // 4, c % 4
        tj = 2044 + l - 4 * jj
        full[b * T + tj] = results[c]["out"]
    return full.reshape(B, T, D)


_NC_CACHE = None


def _get_nc():
    global _NC_CACHE
    if _NC_CACHE is None:
        _NC_CACHE = build_nc()
    return _NC_CACHE


def kernel(**inputs):
    from concourse import bass_utils
    nc = _get_nc()
    in_maps = _host_prep(inputs)
    res = bass_utils.run_bass_kernel_spmd(nc, in_maps,
                                          core_ids=list(range(NC)))
    return _host_assemble(res.results)


if __name__ == "__main__":
    nc = build_nc()
    print("built ok")



# revision 90
# speedup vs baseline: 1.0634x; 1.0634x over previous
"""Trainium2 Bass kernel for nn_Block (attention + soft top-2 MoE), 8-core SPMD.

Sharding:
  - Attention: stride-4 interleaved causal balance.  Core c = 4b+l owns
    batch-b tokens t === l (mod 4), sorted DESCENDING into its 512 qT
    columns, so key slot s (tokens [128s,128s+128)) is needed by exactly
    the first max(256, 512-32s) columns on EVERY core (uniform SPMD
    schedule, ~50% less score/exp work than the rectangular form).  K/V
    are AllGathered in two d-halves (merged k+v buffers; the second AG is
    triggered after the first pass's gathers so it hides behind wave
    compute); causal staircase masks come from the host.  The per-query
    gate bias is constant along the softmax axis, so it cancels.
  - Router logits run in TRUE f32 (fp22-truncated f32r flips near-tied
    top-2 tokens; min prob margin here is 4.5e-6).
  - MoE: expert-parallel, DENSE — every core runs its expert's fp8
    DoubleRow FFN over all 4096 tokens and scales each token's output by
    its routing weight (0 if not routed here); a bf16 ReduceScatter sums
    the <=2 live contributions per token.  No compaction/indirect DMA.
  - The identity "dummy" expert is applied locally by each token's owner.
"""

import sys

if "/opt/trn_rl_repo" not in sys.path:
    sys.path.insert(0, "/opt/trn_rl_repo")

import numpy as np

import concourse.bass as bass
import concourse.mybir as mybir
import concourse.tile as tile
from concourse import bacc
from concourse.masks import make_identity

F32 = mybir.dt.float32
F32R = mybir.dt.float32r
BF16 = mybir.dt.bfloat16
FP8 = mybir.dt.float8e4
I32 = mybir.dt.int32
AF = mybir.ActivationFunctionType
ALU = mybir.AluOpType
DR = mybir.MatmulPerfMode.DoubleRow
W8SCALE = 256.0

B, T, D = 2, 2048, 1024
H, HD = 16, 64
E = 8
NC = 8
N = B * T                  # 4096 tokens
OWN = N // NC              # 512 tokens per core
NCH = 16                   # key slots (128 keys each, token-sorted)
DFF = 4 * D
CAP = 1024                 # expert token capacity per core
VA = 65                    # v columns per head (64 + ones)
EPS = 1e-5
RG = [list(range(NC))]
HG = 4                     # heads per attention head-group

# Balanced-causal slot schedule.  Queries are owned stride-4 interleaved
# (core l of a batch owns tokens t≡l mod 4) and sorted DESCENDING by
# position in qT columns, so slot s (keys [128s,128s+128)) is needed
# exactly by the first 512-32s columns on EVERY core.  Width is padded
# to >=256 to keep f32r matmuls at 1 cycle/row; the mask region covers
# the causal staircase plus the padding.
SLOT_W = []                # (width, mask_start, mask_width, mask_off)
_moff = 0
for _s in range(NCH):
    _wt = 512 - 32 * _s
    _ws = max(256, _wt)
    _st = max(0, _wt - 32)
    SLOT_W.append((_ws, _st, _ws - _st, _moff))
    _moff += _ws - _st
MTOT = _moff               # 1408 mask columns


def r32(x):
    return x.bitcast(F32R)


def build_nc():
    nc = bacc.Bacc("TRN2", target_bir_lowering=False, debug=False,
                   num_devices=NC)

    # ---- I/O ----
    x_own = nc.dram_tensor("x_own", [OWN, D], F32, kind="ExternalInput")
    wqkv = nc.dram_tensor("wqkv", [D, 3 * D], F32R, kind="ExternalInput")
    wproj = nc.dram_tensor("wproj", [D, D], F32R, kind="ExternalInput")
    wrouter = nc.dram_tensor("wrouter", [D, E + 1], F32, kind="ExternalInput")
    wfc8 = nc.dram_tensor("wfc8", [128, 32 * 8 * 128], FP8,
                          kind="ExternalInput")
    wpj8 = nc.dram_tensor("wpj8", [128, 16 * 2 * 1024], FP8,
                          kind="ExternalInput")
    ln2bc = nc.dram_tensor("ln2bc", [128, D], F32, kind="ExternalInput")
    dmask = nc.dram_tensor("dmask", [128, MTOT], F32R, kind="ExternalInput")
    ksel = nc.dram_tensor("ksel", [128, NCH], I32, kind="ExternalInput")
    emask = nc.dram_tensor("emask", [128, 16], F32, kind="ExternalInput")
    out = nc.dram_tensor("out", [OWN, D], F32, kind="ExternalOutput")

    # ---- internal DRAM ----
    agin_kv = [nc.dram_tensor(f"agin_kv{i}", [OWN, 512 + 8 * VA], F32)
               for i in range(2)]
    agout_kv = [nc.dram_tensor(f"agout_kv{i}", [N, 512 + 8 * VA], F32,
                               addr_space="Shared") for i in range(2)]
    agin_h2 = nc.dram_tensor("agin_h2", [OWN, D + 16], BF16)
    agout_h2 = nc.dram_tensor("agout_h2", [N, D + 16], BF16,
                              addr_space="Shared")
    xmid_d = nc.dram_tensor("xmid_d", [OWN, D], F32)
    yt2_d = nc.dram_tensor("yt2_d", [128, 8 * OWN], F32R)
    rsin = nc.dram_tensor("rsin", [N, D], BF16)
    rsout = nc.dram_tensor("rsout", [OWN, D], BF16)

    with tile.TileContext(nc) as tc:
        build_body(nc, tc, locals())
    nc.compile()
    return nc


def build_body(nc, tc, t):
    x_own, wqkv, wproj, wrouter, wfc8, wpj8 = (
        t["x_own"], t["wqkv"], t["wproj"], t["wrouter"], t["wfc8"], t["wpj8"])
    ln2bc, dmask, ksel, emask = (
        t["ln2bc"], t["dmask"], t["ksel"], t["emask"])
    agin_kv, agout_kv = t["agin_kv"], t["agout_kv"]
    agin_h2, agout_h2 = t["agin_h2"], t["agout_h2"]
    xmid_d, yt2_d, rsin, rsout, out = (
        t["xmid_d"], t["yt2_d"], t["rsin"], t["rsout"], t["out"])

    ident_p = tc.alloc_tile_pool(name="ident", bufs=1)
    ident = ident_p.tile([128, 128], F32)
    make_identity(nc, ident[:])

    cst_p = tc.alloc_tile_pool(name="cst", bufs=1)
    ksel_sb = cst_p.tile([128, NCH], I32)
    nc.sync.dma_start(ksel_sb[:], ksel[:, :])
    emask_sb = cst_p.tile([128, 16], F32)
    nc.sync.dma_start(emask_sb[:], emask[:, :])
    ones_cf = cst_p.tile([128, 1], F32)
    nc.vector.memset(ones_cf[:], 1.0)
    ones_c = cst_p.tile([128, 1], F32R)
    nc.scalar.activation(ones_c[:], ones_cf[:], AF.Copy)
    ones_rf = cst_p.tile([1, 128], F32)
    nc.vector.memset(ones_rf[:], 1.0)
    ones_r = cst_p.tile([1, 128], F32R)
    nc.scalar.activation(ones_r[:], ones_rf[:], AF.Copy)
    ones33f = cst_p.tile([33, 64], F32)
    nc.vector.memset(ones33f[:], 1.0)
    ones33 = cst_p.tile([33, 64], F32R)
    nc.scalar.activation(ones33[:], ones33f[:], AF.Copy)
    eps_sb = cst_p.tile([128, 1], F32)
    nc.vector.memset(eps_sb[:], EPS)
    nc.eps_sb = eps_sb
    w8 = cst_p.tile([128, 4], F32)              # identity-expert weight
    wff_p = tc.alloc_tile_pool(name="wff", bufs=1)
    wfcs = wff_p.tile([128, 32, 8, 128], FP8)

    # ------- Phase 1: LN1 + QKV; K/V in d-halves with early AllGathers ----
    ab_p = tc.alloc_tile_pool(name="pAB", bufs=1)
    qT = ab_p.tile([128, 8, OWN], F32R)          # [qdim, mc, tok] (x 1/8)
    with tc.tile_pool(name="p1", bufs=2) as p1, \
         tc.tile_pool(name="p1w", bufs=2) as p1w:
        p1ps_cm = tc.tile_pool(name="p1psA", bufs=2, space="PSUM")
        p1ps = p1ps_cm.__enter__()
        p1kv_cm = tc.tile_pool(name="p1psKV", bufs=1, space="PSUM")
        p1kv = p1kv_cm.__enter__()
        xlnT = p1.tile([128, 8, OWN], F32R, tag="xlnT", bufs=1)
        for tt in range(4):
            xs = p1.tile([128, D], F32, tag="xs")
            nc.sync.dma_start(xs[:], x_own[tt * 128:(tt + 1) * 128, :])
            xln = _layernorm(nc, p1, xs, D)
            for dt in range(8):
                pst = p1ps.tile([128, 128], F32, tag="tp")
                nc.tensor.transpose(pst[:], xln[:, dt * 128:(dt + 1) * 128],
                                    ident[:])
                nc.scalar.activation(xlnT[:, dt, tt * 128:(tt + 1) * 128],
                                     pst[:], AF.Copy)
        # token-major k and v(+ones), one d-half at a time; the half-0 AG
        # fires here, the half-1 AG is triggered inside phase 2 (after the
        # pass-0 gathers) so it hides behind wave compute.
        pks = [p1kv.tile([128, 512], F32, tag=f"pkv{i}", name=f"pkv{i}")
               for i in range(4)]
        for half in range(2):
            for sec in range(2):  # 0 = k, 1 = v
                for dt in range(8):
                    wp = p1w.tile([128, 512], F32R, tag="wkv")
                    base = (1 + sec) * D + half * 512
                    nc.sync.dma_start(
                        wp[:], wqkv[dt * 128:(dt + 1) * 128,
                                    base:base + 512])
                    for tt in range(4):
                        nc.tensor.matmul(
                            pks[tt][:],
                            r32(xlnT[:, dt, tt * 128:(tt + 1) * 128]),
                            r32(wp[:]), start=(dt == 0), stop=(dt == 7))
                for tt in range(4):
                    if sec == 0:
                        ks = p1.tile([128, 512], F32, tag="ko")
                        nc.vector.tensor_copy(ks[:], pks[tt][:])
                        nc.sync.dma_start(
                            agin_kv[half][tt * 128:(tt + 1) * 128, 0:512],
                            ks[:])
                    else:
                        vs = p1.tile([128, 8 * VA], F32, tag="vo")
                        vv = vs[:].rearrange("p (h c) -> p h c", c=VA)
                        nc.vector.memset(vv[:, :, 64:65], 1.0)
                        nc.scalar.activation(
                            vv[:, :, 0:64],
                            pks[tt][:].rearrange("p (h c) -> p h c", c=64),
                            AF.Copy)
                        nc.sync.dma_start(
                            agin_kv[half][tt * 128:(tt + 1) * 128,
                                          512:512 + 8 * VA], vs[:])
            if half == 0:
                nc.gpsimd.collective_compute(
                    "AllGather", ALU.bypass, replica_groups=RG,
                    ins=[agin_kv[0][:, :].opt()],
                    outs=[agout_kv[0][:, :].opt()])
        p1kv_cm.__exit__(None, None, None)
        # qT: lhsT = Wq block -> direct [qdim, tok] layout; scaled 1/sqrt(HD)
        for mc in range(8):
            wp = p1w.tile([128, 8, 128], F32R, tag="wq")
            nc.sync.dma_start(
                wp[:], wqkv[:, mc * 128:(mc + 1) * 128].rearrange(
                    "(dt p) m -> p dt m", p=128))
            psq = p1ps.tile([128, OWN], F32, tag="psq")
            for dt in range(8):
                nc.tensor.matmul(
                    psq[:], r32(wp[:, dt, :]), r32(xlnT[:, dt, :]),
                    start=(dt == 0), stop=(dt == 7))
            nc.scalar.activation(qT[:, mc, :], psq[:], AF.Copy, scale=0.125)
        p1ps_cm.__exit__(None, None, None)

    # FFN weights prefetched on the sync queue; they stream in under the
    # K/V AllGathers and attention waves.
    nc.sync.dma_start(
        wfcs[:], wfc8[:, :].rearrange("p (g dt m) -> p g dt m",
                                      dt=8, m=128))

    # -------- Phase 2: attention (balanced causal slots, two passes) ------
    with tc.tile_pool(name="p2", bufs=1) as p2, \
         tc.tile_pool(name="p2s", bufs=3) as p2s:
        dm_sb = p2.tile([128, MTOT], F32R, tag="dm")
        nc.sync.dma_start(dm_sb[:], dmask[:, :])
        for pz in range(2):     # pass 0: heads 0-7, pass 1: heads 8-15
            kTh = p2.tile([128, NCH, 4, 128], F32R, tag="kTh", bufs=1,
                          name="kTh")
            kvs = []
            tp_cm = tc.tile_pool(name=f"p2psT{pz}", bufs=2, space="PSUM")
            tpp = tp_cm.__enter__()
            for s in range(NCH):
                kvch = p2.tile([128, 512 + 8 * VA], F32R, tag=f"kvch{s}",
                               bufs=1, name=f"kvch{s}")
                kvs.append(kvch)
                nc.gpsimd.indirect_dma_start(
                    out=kvch[:], out_offset=None, in_=agout_kv[pz][:, :],
                    in_offset=bass.IndirectOffsetOnAxis(
                        ap=ksel_sb[:, s:s + 1], axis=0))
                for dtl in range(4):
                    pst = tpp.tile([128, 128], F32, tag="tp2")
                    nc.tensor.transpose(
                        pst[:], kvch[:, dtl * 128:(dtl + 1) * 128].bitcast(
                            F32), ident[:])
                    nc.vector.tensor_copy(kTh[:, s, dtl, :], pst[:])
            if pz == 0:
                # half-1 AllGather rides behind the pass-0 waves
                nc.gpsimd.collective_compute(
                    "AllGather", ALU.bypass, replica_groups=RG,
                    ins=[agin_kv[1][:, :].opt()],
                    outs=[agout_kv[1][:, :].opt()])
            tp_cm.__exit__(None, None, None)
            sc_cm = tc.tile_pool(name=f"p2psS{pz}", bufs=2, space="PSUM")
            scp = sc_cm.__enter__()
            yp_cm = tc.tile_pool(name=f"p2psY{pz}", bufs=1, space="PSUM")
            ypp = yp_cm.__enter__()
            for wave in ([0, 1, 2, 3, 4], [5, 6, 7]):
                ypss = {}
                for wi, hh in enumerate(wave):
                    ypss[hh] = ypp.tile([VA, OWN], F32, tag=f"yps{wi}",
                                        name=f"yps{wi}")
                def do_av(s2, hh2, ex2):
                    ws2 = SLOT_W[s2][0]
                    nc.tensor.matmul(
                        ypss[hh2][:, 0:ws2],
                        kvs[s2][:, 512 + hh2 * VA:512 + (hh2 + 1) * VA],
                        r32(ex2[:, 0:ws2]),
                        start=(s2 == 0), stop=(s2 == NCH - 1),
                        skip_group_check=True)

                # lag-2 software pipeline keeps the PE queue free of
                # exp-dependency stalls (f32r pays 2x when the PE cools)
                pend = []
                for s in range(NCH):
                    ws, mst, mw, moff = SLOT_W[s]
                    for hh in wave:
                        dtl, ph = hh // 2, (hh % 2) * 64
                        ssc = scp.tile([128, OWN], F32, tag="ssc")
                        nc.tensor.matmul(
                            ssc[:, 0:ws],
                            r32(kTh[ph:ph + 64, s, dtl, :]),
                            r32(qT[ph:ph + 64, 4 * pz + dtl, 0:ws]),
                            start=True, stop=True)
                        ex = p2s.tile([128, OWN], F32R, tag="ex")
                        # exp only the true causal width; the padded tail
                        # [wt:ws] keeps stale (finite) values that the mask
                        # multiply zeroes exactly
                        wt = max(32, 512 - 32 * s)
                        nc.scalar.activation(ex[:, 0:wt], ssc[:, 0:wt],
                                             AF.Exp)
                        nc.vector.tensor_mul(ex[:, mst:mst + mw],
                                             ex[:, mst:mst + mw],
                                             dm_sb[:, moff:moff + mw])
                        pend.append((s, hh, ex))
                        if len(pend) > 2:
                            do_av(*pend.pop(0))
                for item in pend:
                    do_av(*item)
                # softmax normalize + write yT2 (pair-batched reciprocals)
                prs = [wave[i:i + 2] for i in range(0, len(wave), 2)]
                rins = {}
                for pi, pr in enumerate(prs):
                    dsb = p2s.tile([33, OWN], F32, tag=f"dsb{pi}", bufs=1,
                                   name=f"dsb{pi}")
                    nc.vector.memset(dsb[:], 1.0)
                    nc.vector.tensor_copy(dsb[0:1, :], ypss[pr[0]][64:65, :])
                    if len(pr) > 1:
                        nc.vector.tensor_copy(dsb[32:33, :],
                                              ypss[pr[1]][64:65, :])
                    rin2 = p2s.tile([33, OWN], F32R, tag=f"rin2{pi}", bufs=1,
                                    name=f"rin2{pi}")
                    with nc.allow_low_precision(reason="f32r bcast rhs"):
                        nc.vector.reciprocal(rin2[:], dsb[:])
                    rins[pr[0]] = (rin2, 0)
                    if len(pr) > 1:
                        rins[pr[1]] = (rin2, 32)
                for hh in wave:
                    h = pz * 8 + hh
                    mc, ph = h // 2, (h % 2) * 64
                    rin2, rb = rins[hh]
                    pbc = scp.tile([128, OWN], F32, tag="ssc")
                    nc.tensor.matmul(pbc[0:64, :],
                                     r32(ones33[rb:rb + 1, :]),
                                     r32(rin2[rb:rb + 1, :]),
                                     start=True, stop=True)
                    pbs = p2s.tile([64, OWN], F32, tag="pbs")
                    nc.scalar.activation(pbs[:], pbc[0:64, :], AF.Copy)
                    yo = p2s.tile([64, OWN], F32R, tag="yo")
                    nc.vector.tensor_mul(yo[:], ypss[hh][0:64, :], pbs[:])
                    nc.sync.dma_start(
                        yt2_d[ph:ph + 64, mc * OWN:(mc + 1) * OWN], yo[:])
            yp_cm.__exit__(None, None, None)
            sc_cm.__exit__(None, None, None)
    ab_p.release()

    # ------------- Phase 3: proj + residual + LN2 + router -------------
    with tc.tile_pool(name="p3", bufs=2) as p3, \
         tc.tile_pool(name="p3w", bufs=2) as p3w:
        p3ps_cm = tc.tile_pool(name="p3psA", bufs=2, space="PSUM")
        p3ps = p3ps_cm.__enter__()
        yT2 = p3.tile([128, 8 * OWN], F32R, tag="yT2")
        nc.sync.dma_start(yT2[:], yt2_d[:, :])
        h2nT = p3.tile([128, 8 * OWN], F32, tag="h2nT")
        for tt in range(4):
            pp = p3ps.tile([128, D], F32, tag="pp")
            for dt in range(8):
                wp = p3w.tile([128, D], F32R, tag="wpj3")
                nc.sync.dma_start(wp[:], wproj[dt * 128:(dt + 1) * 128, :])
                for half in range(2):
                    nc.tensor.matmul(
                        pp[:, half * 512:(half + 1) * 512],
                        r32(yT2[:, dt * OWN + tt * 128:
                                dt * OWN + (tt + 1) * 128]),
                        r32(wp[:, half * 512:(half + 1) * 512]),
                        start=(dt == 0), stop=(dt == 7))
            xot = p3.tile([128, D], F32, tag="xot")
            nc.sync.dma_start(xot[:], x_own[tt * 128:(tt + 1) * 128, :])
            xmt = p3.tile([128, D], F32, tag="xmt")
            nc.vector.tensor_add(xmt[:], xot[:], pp[:])
            nc.sync.dma_start(xmid_d[tt * 128:(tt + 1) * 128, :], xmt[:])
            h2t = _layernorm(nc, p3, xmt, D)
            h2b = p3.tile([128, D], BF16, tag="h2b")
            nc.scalar.activation(h2b[:], h2t[:], AF.Copy)
            nc.sync.dma_start(agin_h2[tt * 128:(tt + 1) * 128, 0:D], h2b[:])
            for dt in range(8):
                pst = p3ps.tile([128, 128], F32, tag="tp3")
                nc.tensor.transpose(pst[:], h2t[:, dt * 128:(dt + 1) * 128],
                                    ident[:])
                nc.scalar.activation(
                    h2nT[:, dt * OWN + tt * 128:dt * OWN + (tt + 1) * 128],
                    pst[:], AF.Copy)
        p3ps_cm.__exit__(None, None, None)
        p3ps_cm2 = tc.tile_pool(name="p3psB", bufs=2, space="PSUM")
        p3ps = p3ps_cm2.__enter__()
        # router logitsT [9, 512]
        wr = p3.tile([128, 8 * (E + 1)], F32, tag="wr")
        nc.sync.dma_start(
            wr[:].rearrange("p (dt m) -> p dt m", m=E + 1),
            wrouter[:, :].rearrange("(dt p) m -> p dt m", p=128))
        plg = p3ps.tile([E + 1, OWN], F32, tag="plg")
        for dt in range(8):
            nc.tensor.matmul(
                plg[:], wr[:, dt * (E + 1):(dt + 1) * (E + 1)],
                h2nT[:, dt * OWN:(dt + 1) * OWN],
                start=(dt == 0), stop=(dt == 7))
        lgT = p3.tile([E + 1, OWN], F32, tag="lgT")
        nc.scalar.activation(lgT[:], plg[:], AF.Copy)
        for tt in range(4):
            plt = p3ps.tile([128, E + 1], F32, tag="plt")
            nc.tensor.transpose(plt[:], lgT[:, tt * 128:(tt + 1) * 128],
                                ident[0:E + 1, 0:E + 1])
            # softmax + top-2 weights on [128, 9]
            lg = p3.tile([128, E + 1], F32, tag="lg")
            nc.vector.tensor_copy(lg[:], plt[:])
            rmax = p3.tile([128, 1], F32, tag="rmax")
            nc.vector.reduce_max(rmax[:], lg[:], axis=mybir.AxisListType.X)
            nrm = p3.tile([128, 1], F32, tag="nrm")
            nc.vector.tensor_scalar_mul(nrm[:], rmax[:], -1.0)
            prob = p3.tile([128, E + 1], F32, tag="prob")
            sume = p3.tile([128, 1], F32, tag="sume")
            nc.scalar.activation(prob[:], lg[:], AF.Exp, bias=nrm[:],
                                 accum_out=sume[:])
            rinv = p3.tile([128, 1], F32, tag="rinv")
            nc.vector.reciprocal(rinv[:], sume[:])
            nc.scalar.activation(prob[:], prob[:], AF.Copy, scale=rinv[:])
            m1 = p3.tile([128, 1], F32, tag="m1")
            nc.vector.reduce_max(m1[:], prob[:], axis=mybir.AxisListType.X)
            eq = p3.tile([128, E + 1], F32, tag="eq")
            nc.vector.tensor_tensor(
                out=eq[:], in0=prob[:], in1=m1[:].to_broadcast([128, E + 1]),
                op=ALU.is_equal)
            pm = p3.tile([128, E + 1], F32, tag="pm")
            nc.vector.tensor_scalar_mul(pm[:], eq[:], -2.0)
            nc.vector.tensor_add(pm[:], pm[:], prob[:])
            m2 = p3.tile([128, 1], F32, tag="m2")
            nc.vector.reduce_max(m2[:], pm[:], axis=mybir.AxisListType.X)
            ge = p3.tile([128, E + 1], F32, tag="ge")
            nc.vector.tensor_tensor(
                out=ge[:], in0=prob[:], in1=m2[:].to_broadcast([128, E + 1]),
                op=ALU.is_ge)
            w16 = p3.tile([128, 16], F32, tag="w16")
            nc.vector.memset(w16[:], 0.0)
            nc.vector.tensor_mul(w16[:, 0:E + 1], prob[:], ge[:])
            nc.vector.tensor_copy(w8[:, tt:tt + 1], w16[:, E:E + 1])
            w16b = p3.tile([128, 16], BF16, tag="w16b")
            nc.vector.tensor_copy(w16b[:], w16[:])
            nc.sync.dma_start(agin_h2[tt * 128:(tt + 1) * 128, D:D + 16],
                              w16b[:])
        p3ps_cm2.__exit__(None, None, None)
    nc.gpsimd.collective_compute(
        "AllGather", ALU.bypass, replica_groups=RG,
        ins=[agin_h2[:, :].opt()], outs=[agout_h2[:, :].opt()])

    # ------- Phase 4/5: dense masked expert FFN (fp8 DoubleRow) ----------
    # Every core runs ITS expert over all N tokens and scales each token's
    # output by that expert's routing weight (0 for tokens not routed here);
    # the ReduceScatter then sums the <=2 live expert contributions.  No
    # compaction, no indirect DMA.
    with tc.tile_pool(name="p5g", bufs=3) as p5g, \
         tc.tile_pool(name="p5", bufs=1) as p5, \
         tc.tile_pool(name="p5at", bufs=1) as p5at:
        identb = p5.tile([128, 128], BF16, tag="identb")
        nc.scalar.activation(identb[:], ident[:], AF.Copy)
        # routing weights for this expert: wcol[p, f] (token p+128f), and
        # fold in the 1/W8SCALE fp8 descale
        wfull = p5.tile([128, 32, 16], BF16, tag="wfull")
        nc.sync.dma_start(
            wfull[:],
            agout_h2[:, D:D + 16].rearrange("(f p) c -> p f c", p=128))
        wsel = p5.tile([128, 32, 16], F32, tag="wsel")
        nc.vector.tensor_tensor(
            out=wsel[:], in0=wfull[:],
            in1=emask_sb[:].rearrange("p (o c) -> p o c", o=1).to_broadcast(
                [128, 32, 16]),
            op=ALU.mult)
        wcol = p5.tile([128, 32], F32, tag="wcol")
        nc.vector.reduce_sum(wcol[:], wsel[:], axis=mybir.AxisListType.X)
        nc.vector.tensor_scalar_mul(wcol[:], wcol[:], 1.0 / W8SCALE)
        wpp = []
        for fp in range(16):
            w2 = p5at.tile([128, 2, D], FP8, tag=f"wpp{fp}",
                           name=f"wpp{fp}")
            nc.sync.dma_start(
                w2[:], wpj8[:, fp * 2048:(fp + 1) * 2048].rearrange(
                    "p (s m) -> p s m", m=1024))
            wpp.append(w2)
        at2 = [p5at.tile([128, 2, CAP], FP8, tag=f"at2_{i}",
                         name=f"at2_{i}") for i in range(16)]
        for q4 in range(4):            # token quarters of 1024
            p5ps_cm = tc.tile_pool(name=f"p5psT{q4}", bufs=2, space="PSUM")
            p5ps = p5ps_cm.__enter__()
            h2cT = p5.tile([128, 8, CAP], FP8, tag="h2cT", bufs=2)
            for j in range(8):
                hc = p5g.tile([128, D], BF16, tag="hc")
                nc.sync.dma_start(
                    hc[:],
                    agout_h2[(q4 * 8 + j) * 128:(q4 * 8 + j + 1) * 128, 0:D])
                for dt in range(8):
                    pst = p5ps.tile([128, 128], BF16, tag="tp5")
                    nc.tensor.transpose(
                        pst[:], hc[:, dt * 128:(dt + 1) * 128], identb[:])
                    nc.scalar.activation(h2cT[:, dt, j * 128:(j + 1) * 128],
                                         pst[:], AF.Copy)
            p5ps_cm.__exit__(None, None, None)
            p5ps_cm2 = tc.tile_pool(name=f"p5psB{q4}", bufs=2, space="PSUM")
            p5ps = p5ps_cm2.__enter__()
            for gfc in range(32):
                ps1 = p5ps.tile([128, CAP], F32, tag="ps1")
                for t2 in range(4):
                    for hf in range(2):
                        nc.tensor.matmul(
                            ps1[:, hf * 512:(hf + 1) * 512],
                            wfcs[:, gfc, 2 * t2:2 * t2 + 2, :],
                            h2cT[:, 2 * t2:2 * t2 + 2,
                                 hf * 512:(hf + 1) * 512],
                            start=(t2 == 0), stop=(t2 == 3), perf_mode=DR)
                nc.scalar.activation(at2[gfc // 2][:, gfc % 2, :], ps1[:],
                                     AF.Gelu, scale=1.0 / W8SCALE)
            p5ps_cm2.__exit__(None, None, None)
            p5ps_cm3 = tc.tile_pool(name=f"p5psC{q4}", bufs=2, space="PSUM")
            p5ps3 = p5ps_cm3.__enter__()
            for tt in range(8):
                ps2 = p5ps3.tile([128, D], F32, tag="ps2")
                for fp in range(16):
                    for hf in range(2):
                        nc.tensor.matmul(
                            ps2[:, hf * 512:(hf + 1) * 512],
                            at2[fp][:, :, tt * 128:(tt + 1) * 128],
                            wpp[fp][:, :, hf * 512:(hf + 1) * 512],
                            start=(fp == 0), stop=(fp == 15), perf_mode=DR)
                sc = p5g.tile([128, D], BF16, tag="sc")
                nc.scalar.activation(sc[:], ps2[:], AF.Copy,
                                     scale=wcol[:, q4 * 8 + tt:
                                                q4 * 8 + tt + 1])
                nc.sync.dma_start(
                    rsin[(q4 * 8 + tt) * 128:(q4 * 8 + tt + 1) * 128, :],
                    sc[:])
            p5ps_cm3.__exit__(None, None, None)

    # ---------------- Phase 6: combine via ReduceScatter ----------------
    nc.gpsimd.collective_compute(
        "ReduceScatter", ALU.add, replica_groups=RG,
        ins=[rsin[:, :].opt()], outs=[rsout[:, :].opt()])

    # ---------------- Phase 7: final assembly ----------------
    with tc.tile_pool(name="p7", bufs=2) as p7:
        lnb = p7.tile([128, D], F32, tag="lnb")
        nc.sync.dma_start(lnb[:], ln2bc[:, :])
        for tt in range(4):
            rs = p7.tile([128, D], BF16, tag="rs")
            nc.sync.dma_start(rs[:], rsout[tt * 128:(tt + 1) * 128, :])
            h2t7 = p7.tile([128, D], BF16, tag="h2t7")
            nc.sync.dma_start(h2t7[:],
                              agin_h2[tt * 128:(tt + 1) * 128, 0:D])
            xm7 = p7.tile([128, D], F32, tag="xm7")
            nc.sync.dma_start(xm7[:], xmid_d[tt * 128:(tt + 1) * 128, :])
            idt = p7.tile([128, D], F32, tag="idt")
            nc.vector.tensor_mul(idt[:], h2t7[:], lnb[:])
            nc.scalar.activation(idt[:], idt[:], AF.Copy,
                                 scale=w8[:, tt:tt + 1])
            nc.vector.tensor_add(idt[:], idt[:], rs[:])
            nc.vector.tensor_add(idt[:], idt[:], xm7[:])
            nc.sync.dma_start(out[tt * 128:(tt + 1) * 128, :], idt[:])
    for pl in (wff_p, cst_p, ident_p):
        pl.release()


def _layernorm(nc, pool, xs, d):
    """LN (no weight) on a [128, d] token-major tile; returns a new tile."""
    rsum = pool.tile([128, 1], F32, tag="ln_rsum")
    nc.vector.reduce_sum(rsum[:], xs[:], axis=mybir.AxisListType.X)
    nmean = pool.tile([128, 1], F32, tag="ln_nmean")
    nc.vector.tensor_scalar_mul(nmean[:], rsum[:], -1.0 / d)
    xc = pool.tile([128, d], F32, tag="ln_xc")
    nc.vector.tensor_scalar_add(xc[:], xs[:], nmean[:])
    ssum = pool.tile([128, 1], F32, tag="ln_ssum")
    nc.scalar.activation(xs[:], xc[:], AF.Square, accum_out=ssum[:])
    std = pool.tile([128, 1], F32, tag="ln_std")
    nc.scalar.activation(std[:], ssum[:], AF.Sqrt, bias=nc.eps_sb[:],
                         scale=1.0 / d)
    rstd = pool.tile([128, 1], F32, tag="ln_rstd")
    nc.vector.reciprocal(rstd[:], std[:])
    xo = pool.tile([128, d], F32, tag="ln_xo")
    nc.scalar.activation(xo[:], xc[:], AF.Copy, scale=rstd[:])
    return xo


# ---------------------------------------------------------------------------
# host side
# ---------------------------------------------------------------------------

def _host_prep(inputs):
    """Build per-core in_maps (all numpy, fp32/int32/fp8)."""
    import ml_dtypes
    f8 = ml_dtypes.float8_e4m3

    x = np.asarray(inputs["x"], np.float32).reshape(N, D)
    ln1 = np.asarray(inputs["ln1_w"], np.float32)
    ln2 = np.asarray(inputs["ln2_w"], np.float32)
    wqkv = (np.asarray(inputs["Wqkv"], np.float32) * ln1[:, None]).copy()
    wproj = np.ascontiguousarray(np.asarray(inputs["Wproj"], np.float32))
    wrouter = (np.asarray(inputs["router_W"], np.float32)
               * ln2[:, None]).copy()
    wfc = np.asarray(inputs["W_fc"], np.float32) * ln2[None, :, None]
    wpj = np.asarray(inputs["W_pj"], np.float32)

    def q8(w):
        return np.clip(w * W8SCALE, -240.0, 240.0).astype(f8)

    # wfc8[e][p, gfc*1024 + dt*128 + m] = wfc[e][dt*128+p, gfc*128+m] * 256
    wfc8 = q8(wfc).reshape(E, 8, 128, 32, 128).transpose(0, 2, 3, 1, 4) \
        .reshape(E, 128, 32 * 8 * 128)
    # wpj8[e][p, fp*2048 + s*1024 + m] = wpj[e][fp*256+s*128+p, m] * 256
    wpj8 = q8(wpj).reshape(E, 16, 2, 128, D).transpose(0, 3, 1, 2, 4) \
        .reshape(E, 128, 16 * 2 * 1024)
    ln2bc = np.broadcast_to(ln2, (128, D)).copy()

    # stride-4 interleaved ownership, queries sorted descending:
    # core c = 4b+l owns batch-b tokens t = 2044 + l - 4j  (j = 0..511)
    jj = np.arange(OWN)
    kk = np.arange(128)[:, None]
    # ksel[k, s] = agout row of batch token 128s+k (same for both batches)
    ktok = 128 * np.arange(NCH)[None, :] + kk          # [128, NCH]
    kown = ktok % 4
    krow = (2044 + kown - ktok) // 4
    ksel_all = (kown * OWN + krow).astype(np.int32)

    in_maps = []
    for c in range(NC):
        b, l = c // 4, c % 4
        tj = 2044 + l - 4 * jj                          # [OWN]
        x_own = x[b * T + tj]
        # causal staircase masks per slot, concatenated mask regions
        dmask = np.zeros((128, MTOT), np.float32)
        for s in range(NCH):
            ws, mst, mw, moff = SLOT_W[s]
            cols = tj[mst:mst + mw][None, :]            # query token ids
            keys = 128 * s + kk                         # [128, 1]
            dmask[:, moff:moff + mw] = (cols >= keys).astype(np.float32)
        em = np.zeros((128, 16), np.float32)
        em[:, c] = 1.0
        in_maps.append({
            "x_own": np.ascontiguousarray(x_own),
            "wqkv": wqkv, "wproj": wproj, "wrouter": wrouter,
            "wfc8": np.ascontiguousarray(wfc8[c]),
            "wpj8": np.ascontiguousarray(wpj8[c]),
            "ln2bc": ln2bc, "dmask": dmask,
            "ksel": ksel_all + np.int32(b * 4 * OWN), "emask": em,
        })
    return in_maps


def _host_assemble(results):
    full = np.empty((N, D), np.float32)
    jj = np.arange(OWN)
    for c in range(NC):
        b, l = c // 4, c % 4
        tj = 2044 + l - 4 * jj
        full[b * T + tj] = results[c]["out"]
    return full.reshape(B, T, D)


_NC_CACHE = None


def _get_nc():
    global _NC_CACHE
    if _NC_CACHE is None:
        _NC_CACHE = build_nc()
    return _NC_CACHE


def kernel(**inputs):
    from concourse import bass_utils
    nc = _get_nc()
    in_maps = _host_prep(inputs)
    res = bass_utils.run_bass_kernel_spmd(nc, in_maps,
                                          core_ids=list(range(NC)))
    return _host_assemble(res.results)


if __name__ == "__main__":
    nc = build_nc()
    print("built ok")

